# revision 1
# baseline (speedup 1.0000x reference)
"""Trainium2 Bass kernel for: out = relu(einsum('bcs,cs->bs', x, w) + bias).

Full shapes: x [32, 2048, 4096] f32, w [2048, 4096] f32, bias [4096] f32.
Sharding: the s-axis (4096) is split across 8 cores (512 each). Each core
reads its x slice (128 MiB) and w/bias slice (4 MiB) once — the minimum
possible HBM traffic — and produces out[:, s_slice]. Gather = concat.

Per-core dataflow (partitions = 128-channel block, free = s):
  DMA   x[b] slice  -> SBUF [128, 16*512]          (4 MiB per batch)
  DVE   xb *= w     (fp32 elementwise, in place)
  PE    ones-matmul per c-block, accumulating the 128-partition reduction
        of each [128, 512] product block into PSUM [1, 512]; the bias row
        is folded in as a K=1 matmul that opens the accumulation group.
  ACT   relu during PSUM -> SBUF copy into out row b
  DMA   out [32, 512] -> DRAM
"""

import numpy as np

B, C, S_FULL = 32, 2048, 4096
N_CORES = 8
S = S_FULL // N_CORES          # 512 s-values per core
P = 128                        # SBUF partitions
CB = C // P                    # 16 channel blocks

# PE reduction dtype: fp32 matmul streams at 4 cyc/row; float32r at 1 cyc/row
# (reduced precision — validated empirically against the fp32 reference).
USE_F32R = False
# First K_FOLD block-pairs are pre-added on DVE to offload the (4x slower)
# fp32 PE reduction. 0 disables. Only meaningful when USE_F32R is False.
K_FOLD = 5
# c-blocks per tile (half-batch granularity keeps the PE fed so its HAM
# clock gate stays warm, and halves the pipeline tail)
HB = CB // 2

_nc_cache = {}


def _build():
    import concourse.bacc as bacc
    import concourse.mybir as mybir
    import concourse.tile as tile

    f32 = mybir.dt.float32
    nc = bacc.Bacc(
        "TRN2",
        target_bir_lowering=False,
        debug=False,
        enable_asserts=False,
        num_devices=N_CORES,
    )

    x = nc.dram_tensor("xs", [B, C, S], f32, kind="ExternalInput").ap()
    w = nc.dram_tensor("ws", [C, S], f32, kind="ExternalInput").ap()
    bias = nc.dram_tensor("bs", [1, S], f32, kind="ExternalInput").ap()
    out = nc.dram_tensor("out", [B, S], f32, kind="ExternalOutput").ap()

    with tile.TileContext(nc) as tc:
        with (
            tc.tile_pool(name="const", bufs=1) as cpool,
            tc.tile_pool(name="xp", bufs=4) as xpool,
            tc.tile_pool(name="pp", bufs=3) as ppool,
            tc.tile_pool(name="ps", bufs=4, space="PSUM") as pspool,
            tc.tile_pool(name="op", bufs=1) as opool,
        ):
            # w/bias lead the Sync ring ahead of the x stream: a second
            # concurrent HWDGE stream (scalar ring) measures ~8% slower
            # per packet, which costs more than the serial weight load.
            w_sb = cpool.tile([P, CB * S], f32)
            nc.sync.dma_start(
                w_sb[:].rearrange("p (cb s) -> p cb s", cb=CB),
                w.rearrange("(cb p) s -> p cb s", p=P),
            )

            # lhsT of the reduction matmuls; float32r requires every matmul
            # input be produced with float32r dtype (rounded on write).
            red_dt = mybir.dt.float32r if USE_F32R else f32
            ones_f32 = cpool.tile([P, 1], f32)
            nc.vector.memset(ones_f32[:], 1.0)
            if USE_F32R:
                # memset can't write float32r; round via DVE copy
                ones = cpool.tile([P, 1], red_dt)
                nc.vector.tensor_copy(ones[:], ones_f32[:])
            else:
                ones = ones_f32

            # scalar ring: keeps this 2 KiB transfer (and its trigger) out
            # of the w -> x0 handoff on the sync ring
            bias_sb = cpool.tile([1, S], f32)
            nc.scalar.dma_start(bias_sb[:], bias[:])

            # Single-partition output staging: compute engines may only
            # address APs with a 32-aligned base partition, so out rows
            # live along the free axis at partition 0. Half-sized and
            # drained twice — the 32 KiB saved lets the x pool hold 4
            # slots, so x loads never wait on PE finishing a batch (the
            # in-place product keeps each slot live through its matmuls).
            HALF = B // 2
            out_sb = opool.tile([1, HALF * S], f32)


            nfold = 0 if USE_F32R else K_FOLD
            x_r = x.rearrange("b (cb p) s -> b p cb s", p=P)
            for b in range(B):
                # One 4 MiB load per batch minimizes per-trigger ring-rearm
                # gaps; the last two batches load in halves so the drain
                # tail after the final transfer is short.
                xb = xpool.tile([P, CB * S], f32, tag="xb")
                xb_r = xb[:].rearrange("p (cb s) -> p cb s", cb=CB)
                # One 4 MiB transfer + one full-tile mul per batch (fewest
                # triggers and DVE instructions; finer slicing mid-stream
                # measured slower). The final batch runs in quarters: with
                # 4 x-buffers its loads no longer wait on PE-held slots
                # (the bufs=3 failure mode), so this genuinely shortens the
                # post-stream chain from ~17 us to ~10 us.
                nchunk = 4 if b == B - 1 else 1
                CH = CB // nchunk
                ps = pspool.tile([1, S], f32)
                # bias fold-in: K=1 matmul opens the accumulation group
                # (plain fp32 — 512 rows, negligible PE time)
                nc.tensor.matmul(
                    ps[:], ones_f32[0:1, 0:1], bias_sb[:], start=True, stop=False
                )
                for h in range(nchunk):
                    r0 = h * CH * S
                    r1 = (h + 1) * CH * S
                    nc.sync.dma_start(
                        xb_r[:, h * CH : (h + 1) * CH, :],
                        x_r[b, :, h * CH : (h + 1) * CH, :],
                    )
                    if USE_F32R:
                        # separate product tile: the verifier's aliasing
                        # analysis rejects in-place rounding between the x
                        # DMA and the fp32r matmul reads
                        prod = ppool.tile([P, CB * S], red_dt, tag="prod")
                        nc.vector.tensor_mul(
                            prod[:, r0:r1], xb[:, r0:r1], w_sb[:, r0:r1]
                        )
                    else:
                        prod = xb
                        nc.vector.tensor_mul(
                            xb[:, r0:r1], xb[:, r0:r1], w_sb[:, r0:r1]
                        )

                    # fold block 2k+1 into block 2k on DVE (offloads the
                    # 4x slower fp32 PE reduction)
                    # One fused fold (blocks[0:kf] += blocks[kf:2kf]) instead
                    # of kf separate adds: same arithmetic and PE matmul
                    # count, but ~150 cycles of DVE issue overhead per
                    # instruction saved — keeps DVE under the DMA pace so
                    # its lag (and the end-of-stream drain) stays constant.
                    kf = nfold // nchunk
                    pbase = h * CH
                    if kf:
                        dst = prod[:, pbase * S : (pbase + kf) * S]
                        src = prod[:, (pbase + kf) * S : (pbase + 2 * kf) * S]
                        nc.vector.tensor_add(dst, dst, src)
                    blocks = list(range(kf)) + list(range(2 * kf, CH))
                    last = h == nchunk - 1
                    for i, cb in enumerate(blocks):
                        j = pbase + cb
                        rhs = prod[:, j * S : (j + 1) * S]
                        nc.tensor.matmul(
                            ps[:],
                            ones[:],
                            rhs,
                            start=False,
                            stop=(last and i == len(blocks) - 1),
                        )

                nc.scalar.activation(
                    out_sb[0:1, (b % HALF) * S : (b % HALF + 1) * S],
                    ps[:],
                    mybir.ActivationFunctionType.Relu,
                )
                if b == HALF - 1:
                    # Scalar ring: on the sync ring this drain's wait-on-ACT
                    # would block later x triggers (FIFO per engine) — a
                    # measured 13 us mid-stream stall.
                    nc.scalar.dma_start(
                        out[0:HALF].unsqueeze(0),
                        out_sb[:].rearrange("p (b s) -> p b s", b=HALF),
                    )

            nc.sync.dma_start(
                out[HALF:].unsqueeze(0),
                out_sb[:].rearrange("p (b s) -> p b s", b=HALF),
            )

    nc.compile()
    return nc


def _get_nc():
    if "nc" not in _nc_cache:
        _nc_cache["nc"] = _build()
    return _nc_cache["nc"]


def _shard_inputs(x, weights, bias):
    x = np.asarray(x)
    weights = np.asarray(weights)
    bias = np.asarray(bias)
    in_maps = []
    for i in range(N_CORES):
        sl = slice(i * S, (i + 1) * S)
        in_maps.append(
            {
                "xs": np.ascontiguousarray(x[:, :, sl], dtype=np.float32),
                "ws": np.ascontiguousarray(weights[:, sl], dtype=np.float32),
                "bs": np.ascontiguousarray(
                    bias[sl].reshape(1, S), dtype=np.float32
                ),
            }
        )
    return in_maps


def _run(inputs, trace=False, trace_cores=None):
    from concourse import bass_utils

    nc = _get_nc()
    in_maps = _shard_inputs(inputs["x"], inputs["weights"], inputs["bias"])
    res = bass_utils.run_bass_kernel_spmd(
        nc,
        in_maps,
        core_ids=list(range(N_CORES)),
        trace=trace,
        trace_cores=trace_cores,
    )
    out = np.concatenate([r["out"] for r in res.results], axis=1)
    return out, res


def kernel(x, weights, bias):
    out, _ = _run({"x": x, "weights": weights, "bias": bias})
    return out



# revision 2
# speedup vs baseline: 1.1540x; 1.1540x over previous
"""Trainium2 Bass kernel for: out = relu(einsum('bcs,cs->bs', x, w) + bias).

Full shapes: x [32, 2048, 4096] f32, w [2048, 4096] f32, bias [4096] f32.
Sharding: the s-axis (4096) is split across 8 cores (512 each). Each core
reads its x slice (128 MiB) and w/bias slice (4 MiB) once — the minimum
possible HBM traffic — and produces out[:, s_slice]. Gather = concat.

The stream is DMA-bound: 132 MiB over 16 DMA engines at 22.5 B/ns each
(360 GB/s aggregate) is ~385 us. Design choices that keep us at that
roofline:

  - p-major channel layout: channel c maps to (partition, k) = (c//16,
    c%16), so each partition's per-batch slice is one contiguous 32 KiB
    DRAM run -> 128 big descriptors per batch instead of 2048 x 2 KiB,
    amortizing the per-packet engine overhead that cost ~5% before.
    (The channel->partition permutation is free: everything is summed.)
  - bf16 products: DVE multiplies x*w writing bf16, so the PE's
    128-partition ones-matmul reduction runs at 1 cyc/row instead of 4.
    PE drops from ~13 us/batch (the old end-of-stream backlog) to ~5,
    well under the ~11.5 us/batch DMA pace even at mid pstate. Rounding
    error is ~2^-9 per product, l2 rel err ~2e-3 vs the f32 reference.
  - one 4 MiB DMA trigger + one full-tile DVE mul per batch (fewest
    triggers; finer mid-stream slicing measured slower on the old
    kernel); the final batch runs in eighths so the post-stream chain
    (last mul -> 2 matmuls -> relu -> drain) is short.
  - per-batch 2 KiB output drains on the scalar ring keep the sync ring
    (x stream) free of waits-on-ACT; bias rides the scalar ring too.

Per-core dataflow (partitions = channel/16, free = k*512 + s):
  DMA   x[b] slice  -> SBUF [128, 8192]             (4 MiB per batch)
  DVE   prod = xb * w  (f32 mul, bf16 write)
  PE    ones-matmul per k-block accumulating the 128-partition reduction
        of each [128, 512] block into PSUM [1, 512]; the bias row is
        folded in as a K=1 f32 matmul that opens the accumulation group.
  ACT   relu during PSUM -> SBUF copy, then a 2 KiB drain to out[b].
"""

import numpy as np

B, C, S_FULL = 32, 2048, 4096
N_CORES = 8
S = S_FULL // N_CORES          # 512 s-values per core
P = 128                        # SBUF partitions
CB = C // P                    # 16 channel blocks per partition
FREE = CB * S                  # 8192 f32 per partition per batch

_nc_cache = {}


def _build():
    import concourse.bacc as bacc
    import concourse.mybir as mybir
    import concourse.tile as tile

    f32 = mybir.dt.float32
    bf16 = mybir.dt.bfloat16
    nc = bacc.Bacc(
        "TRN2",
        target_bir_lowering=False,
        debug=False,
        enable_asserts=False,
        num_devices=N_CORES,
    )

    x = nc.dram_tensor("xs", [B, C, S], f32, kind="ExternalInput").ap()
    w = nc.dram_tensor("ws", [C, S], f32, kind="ExternalInput").ap()
    bias = nc.dram_tensor("bs", [1, S], f32, kind="ExternalInput").ap()
    out = nc.dram_tensor("out", [B, S], f32, kind="ExternalOutput").ap()

    with tile.TileContext(nc) as tc:
        with (
            tc.tile_pool(name="const", bufs=1) as cpool,
            tc.tile_pool(name="xp", bufs=4) as xpool,
            tc.tile_pool(name="pp", bufs=2) as ppool,
            tc.tile_pool(name="ps", bufs=4, space="PSUM") as pspool,
            tc.tile_pool(name="op", bufs=2) as opool,
        ):
            # w leads the sync ring ahead of the x stream (same contiguous
            # p-major layout as x; a second concurrent stream for w measured
            # slower on the old kernel than the serial load).
            w_sb = cpool.tile([P, FREE], f32)
            nc.sync.dma_start(w_sb[:], w.rearrange("(p k) s -> p (k s)", p=P))

            ones_f32 = cpool.tile([P, 1], f32)
            nc.vector.memset(ones_f32[:], 1.0)
            ones_bf = cpool.tile([P, 1], bf16)
            nc.vector.tensor_copy(ones_bf[:], ones_f32[:])

            # scalar ring: keeps this 2 KiB transfer (and its trigger) out
            # of the w -> x0 handoff on the sync ring
            bias_sb = cpool.tile([1, S], f32)
            nc.scalar.dma_start(bias_sb[:], bias[:])

            x_r = x.rearrange("b (p k) s -> b p (k s)", p=P)
            for b in range(B):
                xb = xpool.tile([P, FREE], f32, tag="xb")
                prod = ppool.tile([P, FREE], bf16, tag="prod")
                nchunk = 8 if b == B - 1 else 1
                CH = CB // nchunk
                ps = pspool.tile([1, S], f32)
                # bias fold-in: K=1 matmul opens the accumulation group
                nc.tensor.matmul(
                    ps[:], ones_f32[0:1, 0:1], bias_sb[:], start=True, stop=False
                )
                for h in range(nchunk):
                    r0 = h * CH * S
                    r1 = (h + 1) * CH * S
                    nc.sync.dma_start(xb[:, r0:r1], x_r[b, :, r0:r1])
                    nc.vector.tensor_mul(
                        prod[:, r0:r1], xb[:, r0:r1], w_sb[:, r0:r1]
                    )
                    last = h == nchunk - 1
                    for i in range(CH):
                        j = h * CH + i
                        nc.tensor.matmul(
                            ps[:],
                            ones_bf[:],
                            prod[:, j * S : (j + 1) * S],
                            start=False,
                            stop=(last and i == CH - 1),
                        )

                ob = opool.tile([1, S], f32, tag="ob")
                nc.scalar.activation(
                    ob[:], ps[:], mybir.ActivationFunctionType.Relu
                )
                # per-batch 2 KiB drain on the scalar ring (ACT queue): off
                # the sync ring so x triggers never wait on ACT.
                nc.scalar.dma_start(out[b : b + 1], ob[:])

    nc.compile()
    return nc


def _get_nc():
    if "nc" not in _nc_cache:
        _nc_cache["nc"] = _build()
    return _nc_cache["nc"]


def _shard_inputs(x, weights, bias):
    x = np.asarray(x)
    weights = np.asarray(weights)
    bias = np.asarray(bias)
    in_maps = []
    for i in range(N_CORES):
        sl = slice(i * S, (i + 1) * S)
        in_maps.append(
            {
                "xs": np.ascontiguousarray(x[:, :, sl], dtype=np.float32),
                "ws": np.ascontiguousarray(weights[:, sl], dtype=np.float32),
                "bs": np.ascontiguousarray(
                    bias[sl].reshape(1, S), dtype=np.float32
                ),
            }
        )
    return in_maps


def _run(inputs, trace=False, trace_cores=None):
    from concourse import bass_utils

    nc = _get_nc()
    in_maps = _shard_inputs(inputs["x"], inputs["weights"], inputs["bias"])
    res = bass_utils.run_bass_kernel_spmd(
        nc,
        in_maps,
        core_ids=list(range(N_CORES)),
        trace=trace,
        trace_cores=trace_cores,
    )
    out = np.concatenate([r["out"] for r in res.results], axis=1)
    return out, res


def kernel(x, weights, bias):
    out, _ = _run({"x": x, "weights": weights, "bias": bias})
    return out


# revision 3
# speedup vs baseline: 1.2986x; 1.1253x over previous
"""Trainium2 Bass kernel for: out = relu(einsum('bcs,cs->bs', x, w) + bias).

Full shapes: x [32, 2048, 4096] f32, w [2048, 4096] f32, bias [4096] f32.
Sharding: the s-axis (4096) is split across 8 cores (512 each). Each core
reads its x slice (128 MiB) and w/bias slice (4 MiB) once — the minimum
possible HBM traffic — and produces out[:, s_slice]. Gather = concat.

The stream is DMA-bound. Measured engine behavior (packet trace): the 16
DMA engines move 32 KiB descriptors at ~27 B/ns each (~432 GB/s
aggregate), but each HWDGE queue is serial per trigger — transfer
(~9.7 us/batch) plus ~2.6 us of turnaround (trigger parse, DGE start
delay, straggler-engine semaphore post) before its next transfer starts.
On one queue that turnaround is dead time (measured 12.3 us/batch pace,
~340 GB/s). Design:

  - p-major channel layout: channel c maps to (partition, k) = (c//16,
    c%16), so each partition's per-batch slice is one contiguous 32 KiB
    DRAM run -> 128 big descriptors per batch instead of 2048 x 2 KiB.
  - dual-queue alternation: even batches trigger on the sync ring, odd
    batches on the scalar ring, so one queue's turnaround hides inside
    the other queue's transfer and the engines stay saturated.
  - output drains ride the gpsimd (SWDGE) queue and the relu for batch
    b is emitted after batch b+2's trigger, so neither ring's DGE ever
    sits behind a compute wait.
  - bf16 products: DVE multiplies x*w writing bf16, so the PE's
    128-partition ones-matmul reduction runs at 1 cyc/row instead of 4
    (~5 us/batch worst case vs the old 13 us/batch fp32 backlog).
    Rounding is ~2^-9 per product; l2 rel err ~2e-3 vs f32 reference.
  - the final batch streams in eighths (alternating queues) so the
    post-stream chain (last mul -> 2 matmuls -> relu -> drain) is short.

Per-core dataflow (partitions = channel/16, free = k*512 + s):
  DMA   x[b] slice  -> SBUF [128, 8192]             (4 MiB per batch)
  DVE   prod = xb * w  (f32 mul, bf16 write)
  PE    ones-matmul per k-block accumulating the 128-partition reduction
        of each [128, 512] block into PSUM [1, 512]; the bias row is
        folded in as a K=1 f32 matmul that opens the accumulation group.
  ACT   relu during PSUM -> SBUF copy; GPSIMD drains 2 KiB to out[b].
"""

import numpy as np

B, C, S_FULL = 32, 2048, 4096
N_CORES = 8
S = S_FULL // N_CORES          # 512 s-values per core
P = 128                        # SBUF partitions
CB = C // P                    # 16 channel blocks per partition
FREE = CB * S                  # 8192 f32 per partition per batch

_nc_cache = {}


def _build():
    import concourse.bacc as bacc
    import concourse.mybir as mybir
    import concourse.tile as tile

    f32 = mybir.dt.float32
    bf16 = mybir.dt.bfloat16
    nc = bacc.Bacc(
        "TRN2",
        target_bir_lowering=False,
        debug=False,
        enable_asserts=False,
        num_devices=N_CORES,
    )

    x = nc.dram_tensor("xs", [B, C, S], f32, kind="ExternalInput").ap()
    w = nc.dram_tensor("ws", [C, S], f32, kind="ExternalInput").ap()
    bias = nc.dram_tensor("bs", [1, S], f32, kind="ExternalInput").ap()
    out = nc.dram_tensor("out", [B, S], f32, kind="ExternalOutput").ap()

    with tile.TileContext(nc) as tc:
        with (
            tc.tile_pool(name="const", bufs=1) as cpool,
            tc.tile_pool(name="xp", bufs=4) as xpool,
            tc.tile_pool(name="pp", bufs=2) as ppool,
            tc.tile_pool(name="ps", bufs=6, space="PSUM") as pspool,
            tc.tile_pool(name="op", bufs=2) as opool,
        ):
            # w leads the sync ring ahead of the even x batches; the odd
            # batches start concurrently on the scalar ring.
            w_sb = cpool.tile([P, FREE], f32)
            nc.sync.dma_start(w_sb[:], w.rearrange("(p k) s -> p (k s)", p=P))

            ones_f32 = cpool.tile([P, 1], f32)
            nc.vector.memset(ones_f32[:], 1.0)
            ones_bf = cpool.tile([P, 1], bf16)
            nc.vector.tensor_copy(ones_bf[:], ones_f32[:])

            bias_sb = cpool.tile([1, S], f32)
            nc.scalar.dma_start(bias_sb[:], bias[:])

            x_r = x.rearrange("b (p k) s -> b p (k s)", p=P)
            pending = []  # (b, ps, ob) awaiting relu+drain emission

            def flush_one():
                pb, pps, pob = pending.pop(0)
                nc.scalar.activation(
                    pob[:], pps[:], mybir.ActivationFunctionType.Relu
                )
                # 2 KiB drain on the gpsimd SWDGE queue: keeps both HW
                # rings' DGEs free of drain turnarounds.
                nc.gpsimd.dma_start(out[pb : pb + 1], pob[:])

            for b in range(B):
                ring = nc.sync if b % 2 == 0 else nc.scalar
                xb = xpool.tile([P, FREE], f32, tag="xb")
                prod = ppool.tile([P, FREE], bf16, tag="prod")
                nchunk = 8 if b == B - 1 else 1
                CH = CB // nchunk
                ps = pspool.tile([1, S], f32)
                # bias fold-in: K=1 matmul opens the accumulation group
                nc.tensor.matmul(
                    ps[:], ones_f32[0:1, 0:1], bias_sb[:], start=True, stop=False
                )
                for h in range(nchunk):
                    r0 = h * CH * S
                    r1 = (h + 1) * CH * S
                    cring = ring if nchunk == 1 else (
                        nc.sync if h % 2 == 0 else nc.scalar
                    )
                    cring.dma_start(xb[:, r0:r1], x_r[b, :, r0:r1])
                    nc.vector.tensor_mul(
                        prod[:, r0:r1], xb[:, r0:r1], w_sb[:, r0:r1]
                    )
                    last = h == nchunk - 1
                    for i in range(CH):
                        j = h * CH + i
                        nc.tensor.matmul(
                            ps[:],
                            ones_bf[:],
                            prod[:, j * S : (j + 1) * S],
                            start=False,
                            stop=(last and i == CH - 1),
                        )

                ob = opool.tile([1, S], f32, tag="ob")
                pending.append((b, ps, ob))
                # defer relu/drain 2 batches so the scalar ring's next x
                # trigger is never queued behind a wait-on-PE
                if len(pending) > 2:
                    flush_one()
            while pending:
                flush_one()

    nc.compile()
    return nc


def _get_nc():
    if "nc" not in _nc_cache:
        _nc_cache["nc"] = _build()
    return _nc_cache["nc"]


def _shard_inputs(x, weights, bias):
    x = np.asarray(x)
    weights = np.asarray(weights)
    bias = np.asarray(bias)
    in_maps = []
    for i in range(N_CORES):
        sl = slice(i * S, (i + 1) * S)
        in_maps.append(
            {
                "xs": np.ascontiguousarray(x[:, :, sl], dtype=np.float32),
                "ws": np.ascontiguousarray(weights[:, sl], dtype=np.float32),
                "bs": np.ascontiguousarray(
                    bias[sl].reshape(1, S), dtype=np.float32
                ),
            }
        )
    return in_maps


def _run(inputs, trace=False, trace_cores=None):
    from concourse import bass_utils

    nc = _get_nc()
    in_maps = _shard_inputs(inputs["x"], inputs["weights"], inputs["bias"])
    res = bass_utils.run_bass_kernel_spmd(
        nc,
        in_maps,
        core_ids=list(range(N_CORES)),
        trace=trace,
        trace_cores=trace_cores,
    )
    out = np.concatenate([r["out"] for r in res.results], axis=1)
    return out, res


def kernel(x, weights, bias):
    out, _ = _run({"x": x, "weights": weights, "bias": bias})
    return out
